# revision 1
# baseline (speedup 1.0000x reference)
"""Trainium2 Bass kernel for the ergodicity loss (product-column scheme).

Math: for x[T=512, B=16, N=32, d=2] in [0,1]^2 and modes (k0,k1) in {0..9}^2:
    basis = cos(pi*k0*x0) * cos(pi*k1*x1)                    (separable)
    coeffs[b, k0, k1] = sum_{t,n} basis / (T*N) / nf[k1]
    loss = mean((nw * (coeffs - cd))**2)

Device strategy (8 cores, data-parallel over T: 64 timesteps/core):
  Per core: 2048 points per batch as 128 partitions x 16 chunks, both
  coordinate dims side by side (xx[p, dd*256 + c*16 + b]).

  Mode columns (bf16) hold, per mode k, a KNOWN LINEAR COMBINATION of
  the true cosines c_j = cos(pi*j*x):
    - w_k = x*(k/2) + 128.75 (f32 TS) for generators k in {1,2,4,5}:
      the exponent is pinned to 7, so the low 16 mantissa bits hold
      (x*k/2 + 0.75)*2^16 in fixed point; the signed int16 lane read IS
      the mod-2^16 range reduction (absorbed by sin periodicity).
      (int16 ALU ops saturate on real HW, so the multiply-by-k must
      happen in f32 via this exponent trick, NOT in int16.)
    - planes {1,2,4,5} = Sin(low16(w_k) * -2pi/2^16) = +c_k  (ACT, 2
      batched instructions; arg always in (-pi, pi] by construction)
    - plane 3 = plane1*plane2 = c1c2 = (c3+c1)/2       (DVE TT, bf16)
      plane 6 = plane2*plane4 = (c6+c2)/2
      plane 7 = plane2*plane5 = (c7+c3)/2
      plane 9 = plane4*plane5 = (c9+c1)/2
      plane 8 = plane4*plane4 = (c8+c0)/2              (Pool TT)
      plane 0 = +1 = c0                                (Pool memset)
  The mode-k columns are thus M @ c for a constant lower-triangular M;
  the PSUM Gram matrix S~ = M S M^T is unmixed on the host with M^-1
  (exact linear algebra, fp64).

  PE: per (chunk c in 16, batch-group g in 2): lhsT = dim-0 columns
  [k:10 x b:8] (2D strided AP), rhs = dim-1 columns; accumulate into
  PSUM[80, 80] over the 16 chunks. Off-diagonal batch blocks unused.

  The activation-table load (1283 ns) is hoisted out of the bench loop
  by a pre-loop warmup Sin whose output is the (zero) bias tensor that
  in-loop activations consume.
Host: sum 8 per-core [80,160] PSUM dumps, extract diagonal batch
blocks, S = Minv S~ Minv^T, then the tiny [16,100] weighted MSE.
"""
import numpy as np

T, B, NA, D = 512, 16, 32, 2
KMAX = 10
NCORES = 8
TLOC = T // NCORES          # 64 timesteps per core
KN = KMAX * KMAX

# derived planes: k -> (i, j) with plane_k = plane_i * plane_j (PLANE indices,
# in dependency order: second-order products use earlier product planes --
# products of linear combinations of cosines stay linear combinations)
PROD = {3: (1, 2), 5: (1, 4), 7: (3, 4)}               # DVE
POOL_PROD = {6: (2, 4), 8: (4, 4), 9: (5, 4)}          # Pool
GENS = (1, 2, 4)

_STATE = {}

CFG = {"unroll": 32, "psum_dma": False, "bufs": 5, "pbufs": 3, "skip": (),
       "fp8_dr": False}


def _np_constants():
    """Replicates reference._constants() exactly in numpy (L = ones)."""
    L = np.ones(D, dtype=np.float32)
    grids = np.meshgrid(*[np.arange(KMAX) for _ in range(D)], indexing="ij")
    K = np.stack(grids, -1).reshape(-1, D).astype(np.float32)          # [100, 2]
    k_scaled = K * np.pi / L
    nf = np.where(K[:, -1] != 0, np.sqrt(L[-1] / 2.0), 1.0).astype(np.float32)
    nw = ((1.0 + (k_scaled ** 2).sum(-1)) ** (-(D + 1) / 2.0) * 100.0).astype(np.float32)
    safe_k = np.where(K != 0, k_scaled, 1.0)
    term = np.where(K != 0,
                    (np.exp(1j * k_scaled * L) - 1.0) / (1j * safe_k * L),
                    1.0 + 0j)
    cd = (term.prod(-1).real / nf).astype(np.float32)                  # [100]
    return nf, nw, cd


def _build_M():
    """M[k] = stored plane k as a linear combination of c_0..c_9.

    Product planes are expanded symbolically: (sum_m a_m c_m)(sum_n b_n c_n)
    = sum a_m b_n (c_{m+n} + c_{|m-n|})/2.  Iteration order of PROD then
    POOL_PROD is the dependency order of the plane DAG.
    """
    M = np.zeros((KMAX, KMAX))
    M[0, 0] = 1.0
    for k in GENS:
        M[k, k] = 1.0
    for k, (i, j) in {**PROD, **POOL_PROD}.items():
        row = np.zeros(KMAX)
        for m in range(KMAX):
            for n in range(KMAX):
                a = M[i, m] * M[j, n]
                if a:
                    row[m + n] += a / 2
                    row[abs(m - n)] += a / 2
        M[k] = row
    return M


def _build(reps: int = 1, loop: bool = False, cfg: dict | None = None):
    import concourse.tile as tile
    from concourse import bacc, mybir

    cfg = {**CFG, **(cfg or {})}
    f32 = mybir.dt.float32
    i16 = mybir.dt.int16
    bf16 = mybir.dt.bfloat16
    AF = mybir.ActivationFunctionType
    OP = mybir.AluOpType

    fp8 = mybir.dt.float8e4
    cdt = fp8 if cfg["fp8_dr"] else bf16

    nc = bacc.Bacc("TRN2", target_bir_lowering=False, debug=False)
    xx = nc.dram_tensor("xx", [128, 512], f32, kind="ExternalInput").ap()
    sout = nc.dram_tensor("sout", [80, 160], f32, kind="ExternalOutput").ap()

    unroll = cfg["unroll"] if loop else 1
    if loop:
        assert reps % unroll == 0, (reps, unroll)

    with tile.TileContext(nc) as tc:
        with tc.tile_pool(name="cpool", bufs=1) as cpool, \
             tc.tile_pool(name="pool", bufs=cfg["bufs"]) as pool, \
             tc.tile_pool(name="ppool", bufs=cfg["pbufs"],
                          space="PSUM") as ppool:
            scale_t = cpool.tile([128, 1], f32)
            bias_t = cpool.tile([128, 1], f32)
            zero_t = cpool.tile([128, 1], f32)
            nc.vector.memset(scale_t[:], -2.0 * float(np.pi) / (1 << 16))
            nc.vector.memset(zero_t[:], 0.0)
            # warmup: loads the Sin table outside the loop; body activations
            # depend on bias_t ( = sin(0) = 0 ) so this cannot sink.
            nc.scalar.activation(bias_t[:], zero_t[:], AF.Sin,
                                 bias=0.0, scale=1.0)

            def body(_i=None):
                XX = pool.tile([128, 512], f32, tag="XX")
                U = pool.tile([128, 512 * len(GENS)], f32, tag="U")
                C = pool.tile([128, 5120], cdt, tag="C")
                # layouts (q = dd*32 + c*2 + g indexes the 64 matmul blocks):
                #   XX col = q*8 + b'            (= dd*256 + c*16 + g*8 + b')
                #   U  col = j*512 + q*8 + b'    (j: generator index, f32)
                #   C  col = q*80 + k*8 + b'     (k: mode)
                # so every matmul operand C[:, q*80 : q*80+80] is contiguous
                # while per-plane APs stay uniform 2-d strides.
                Ub = U[:].bitcast(i16).rearrange(
                    "p (j q b t) -> p q j b t",
                    j=len(GENS), q=64, b=8, t=2)[..., 0]
                Cb = C[:].rearrange("p (q k b) -> p q k b", q=64, k=KMAX, b=8)

                skip = cfg["skip"]
                nc.sync.dma_start(XX[:], xx)
                # w_k = x*(k/2) + 128.75: low int16 mantissa lane holds the
                # phase (x*k/2 + 0.75)*2^16 mod 2^16 (exponent pinned at 7)
                if "ts" not in skip:
                    for j, k in enumerate(GENS):
                        nc.vector.tensor_scalar(U[:, j * 512:(j + 1) * 512],
                                                XX[:], 0.5 * k, 128.75,
                                                OP.mult, OP.add)
                if "sins" not in skip:
                    # planes {1,2} and {4}: +cos(pi*k*x)
                    nc.scalar.activation(Cb[:, :, 1:3], Ub[:, :, 0:2], AF.Sin,
                                         bias=bias_t[:], scale=scale_t[:])
                    nc.scalar.activation(Cb[:, :, 4:5], Ub[:, :, 2:3], AF.Sin,
                                         bias=bias_t[:], scale=scale_t[:])
                if "tt" not in skip:
                    # product planes (DVE)
                    for k, (i, j) in PROD.items():
                        nc.vector.tensor_tensor(Cb[:, :, k], Cb[:, :, i],
                                                Cb[:, :, j], OP.mult)
                if "pool" not in skip:
                    # plane 8 + plane 0 on the (otherwise idle) Pool engine
                    for k, (i, j) in POOL_PROD.items():
                        nc.gpsimd.tensor_tensor(Cb[:, :, k], Cb[:, :, i],
                                                Cb[:, :, j], OP.mult)
                    nc.gpsimd.memset(Cb[:, :, 0], 1.0)

                ps = [ppool.tile([80, 80], f32, name=f"ps{g}", tag=f"ps{g}")
                      for g in range(2)]
                if "pe" not in skip:
                    if cfg["fp8_dr"]:
                        # DoubleRow: one matmul contracts a chunk PAIR
                        # (q-order is g-major: q = dd*32 + g*16 + c)
                        for t in range(8):
                            for g in range(2):
                                q0 = g * 16 + 2 * t
                                lhsT = C[:, q0 * 80:(q0 + 2) * 80].rearrange(
                                    "p (r c) -> p r c", r=2)
                                rhs = C[:, (32 + q0) * 80:(34 + q0) * 80
                                        ].rearrange("p (r c) -> p r c", r=2)
                                nc.tensor.matmul(
                                    ps[g][:], lhsT, rhs,
                                    start=(t == 0), stop=(t == 7),
                                    perf_mode=mybir.MatmulPerfMode.DoubleRow)
                    else:
                        for c in range(16):
                            for g in range(2):
                                q0 = c * 2 + g
                                nc.tensor.matmul(
                                    ps[g][:],
                                    C[:, q0 * 80:q0 * 80 + 80],
                                    C[:, (32 + q0) * 80:(32 + q0) * 80 + 80],
                                    start=(c == 0), stop=(c == 15))
                if "out" not in skip:
                    SO = pool.tile([80, 160], f32, tag="SO")
                    # split PSUM evacuation across ACT and DVE
                    nc.scalar.copy(SO[0:80, 0:80], ps[0][:])
                    nc.vector.tensor_copy(SO[0:80, 80:160], ps[1][:])
                    nc.sync.dma_start(sout, SO[0:80, :])

            if loop:
                with tc.For_i(0, reps // unroll, 1) as i:
                    for _ in range(unroll):
                        body(i)
            else:
                for _ in range(reps):
                    body()

    nc.compile()
    return nc


def _get_state():
    if "nc" not in _STATE:
        _STATE["nc"] = _build()
    return _STATE["nc"]


def _shard_inputs(x: np.ndarray, fp8_dr: bool | None = None):
    """x [512, 16, 32, 2] -> per-core {xx [128, 512]}.

    xx free layout (b = g*8 + b', batch-group g in 2):
      default: dd*256 + c*16 + g*8 + b'   (q-blocks ordered (c, g))
      fp8_dr:  dd*256 + g*128 + c*8 + b'  (g-major, so DoubleRow chunk
               pairs (c, c+1) of one group are adjacent)
    partition p = tp*32 + a for timestep-subgroup tp in 4, agent a in 32.
    """
    if fp8_dr is None:
        fp8_dr = CFG["fp8_dr"]
    in_maps = []
    for core in range(NCORES):
        xc = x[core * TLOC:(core + 1) * TLOC]      # [64, 16, 32, 2]
        arr = xc.reshape(16, 4, 2, 8, 32, 2)       # (c, tp, g, b', a, d)
        if fp8_dr:
            arr = arr.transpose(5, 1, 4, 2, 0, 3)  # (d, tp, a, g, c, b')
        else:
            arr = arr.transpose(5, 1, 4, 0, 2, 3)  # (d, tp, a, c, g, b')
        arr = arr.reshape(2, 128, 256)
        xxc = np.concatenate([arr[0], arr[1]], axis=1)
        in_maps.append({"xx": np.ascontiguousarray(xxc)})
    return in_maps


def _gather(souts):
    """souts: list of 8 [80, 160] partials -> scalar loss (float32).

    sout row = k0*8 + b', col (80*g + k1*8 + b'') for batch b = 8*g + b'.
    """
    total = np.zeros((80, 160), dtype=np.float64)
    for s in souts:
        total += s.astype(np.float64)
    St = np.empty((B, KMAX, KMAX), dtype=np.float64)
    for g in range(2):
        for bp in range(8):
            St[8 * g + bp] = total[bp::8, 80 * g + bp:80 * (g + 1):8]
    Minv = np.linalg.inv(_build_M())
    S = np.einsum("ki,bij,lj->bkl", Minv, St, Minv)
    nf, nw, cd = _np_constants()
    coeffs = S.reshape(B, KN) / (NA * T) / nf[None, :].astype(np.float64)
    d = nw[None, :].astype(np.float64) * (coeffs - cd[None, :].astype(np.float64))
    loss = np.mean(d * d)
    return np.float32(loss)


def kernel(x: np.ndarray) -> np.ndarray:
    from concourse.bass_utils import run_bass_kernel_spmd

    nc = _get_state()
    in_maps = _shard_inputs(np.asarray(x, dtype=np.float32))
    res = run_bass_kernel_spmd(nc, in_maps, list(range(NCORES)))
    souts = [r["sout"] for r in res.results]
    return _gather(souts)



# revision 2
# speedup vs baseline: 1.3173x; 1.3173x over previous
"""Trainium2 Bass kernel for the ergodicity loss (Chebyshev-cascade scheme).

Math: for x[T=512, B=16, N=32, d=2] in [0,1]^2 and modes (k0,k1) in {0..9}^2:
    basis = cos(pi*k0*x0) * cos(pi*k1*x1)                    (separable)
    coeffs[b, k0, k1] = sum_{t,n} basis / (T*N) / nf[k1]
    loss = mean((nw * (coeffs - cd))**2)

Device strategy (8 cores, data-parallel over T: 64 timesteps/core):
  Per core: 2048 points per batch as 128 partitions x 16 chunks, both
  coordinate dims side by side (xx[p, dd*256 + c*16 + b]).

  Mode columns (bf16) hold, per mode k, a KNOWN LINEAR COMBINATION of
  the true cosines c_j = cos(pi*j*x), built WITHOUT any range-reduction
  tricks via a Chebyshev product cascade seeded by one direct Sin:
    - p1 = Sin(-pi*x + pi/2) = cos(pi*x) = c1   (arg in (-pi/2, pi/2],
      inside the HW Sin table domain; ACT, reads xx f32 directly)
    - p2 = Square(sqrt2*p1) = 2*c1^2 = 1 + c2             (ACT)
    - p4 = Square(sqrt2*p2) = c4 + 4c2 + 3                (ACT)
    - p6 = p2*p4, p8 = p4*p4                              (DVE TT)
    - [p3,p5,p7,p9] = [p2,p4,p6,p8] * broadcast(p1): ONE batched DVE TT
      (stride-16 plane APs + stride-0 broadcast of p1)
    - p0 = 1                                              (Pool memset)
  The mode-k columns are thus M @ c for a constant triangular M
  (|Minv|max = 60; high-mode noise amplification is crushed by the
  k^-3 loss weights nw); the PSUM Gram matrix S~ = M S M^T is unmixed
  on the host with M^-1 (exact linear algebra, fp64).

  PE: per (chunk c in 16, batch-group g in 2): lhsT = dim-0 columns
  [k:10 x b:8] (contiguous 80-col block), rhs = dim-1 columns;
  accumulate into PSUM[80, 80] over the 16 chunks. Off-diagonal batch
  blocks unused.

  The activation-table load (1283 ns) is hoisted out of the bench loop
  by a pre-loop warmup Sin whose (zero) output feeds the bias tile
  every in-loop activation consumes.
Host: sum 8 per-core [80,160] PSUM dumps, extract diagonal batch
blocks, S = Minv S~ Minv^T, then the tiny [16,100] weighted MSE.
"""
import numpy as np

T, B, NA, D = 512, 16, 32, 2
KMAX = 10
NCORES = 8
TLOC = T // NCORES          # 64 timesteps per core
KN = KMAX * KMAX

SQRT2 = float(np.sqrt(2.0))

# plane construction DAG (dependency order):
#   ('sin',)            plane = cos(pi x)
#   ('square', src)     plane = (sqrt2 * p_src)^2
#   ('mul', i, j)       plane = p_i * p_j          (DVE TT)
#   ('one',)            plane = 1                  (Pool memset)
# [p3,p5,p7,p9] = [p2,p4,p6,p8]*p1 is emitted as ONE batched TT.
SCHEME = {
    1: ("sin",),
    2: ("square", 1),
    4: ("square", 2),
    6: ("mul", 2, 4),
    8: ("mul", 4, 4),
    3: ("mul", 2, 1),
    5: ("mul", 4, 1),
    7: ("mul", 6, 1),
    9: ("mul", 8, 1),
    0: ("one",),
}

_STATE = {}

CFG = {"unroll": 32, "psum_dma": False, "bufs": 5, "pbufs": 3, "skip": (),
       "batched_tt": True, "pool_p8": False}


def _np_constants():
    """Replicates reference._constants() exactly in numpy (L = ones)."""
    L = np.ones(D, dtype=np.float32)
    grids = np.meshgrid(*[np.arange(KMAX) for _ in range(D)], indexing="ij")
    K = np.stack(grids, -1).reshape(-1, D).astype(np.float32)          # [100, 2]
    k_scaled = K * np.pi / L
    nf = np.where(K[:, -1] != 0, np.sqrt(L[-1] / 2.0), 1.0).astype(np.float32)
    nw = ((1.0 + (k_scaled ** 2).sum(-1)) ** (-(D + 1) / 2.0) * 100.0).astype(np.float32)
    safe_k = np.where(K != 0, k_scaled, 1.0)
    term = np.where(K != 0,
                    (np.exp(1j * k_scaled * L) - 1.0) / (1j * safe_k * L),
                    1.0 + 0j)
    cd = (term.prod(-1).real / nf).astype(np.float32)                  # [100]
    return nf, nw, cd


def _build_M():
    """M[k] = stored plane k as a linear combination of c_0..c_9.

    Symbolic expansion over the Chebyshev product rule:
    c_m * c_n = (c_{m+n} + c_{|m-n|}) / 2 (terms beyond c_9 are dropped:
    the construction never produces them for k<=9 products used here).
    """
    def mul(a, b):
        out = np.zeros(KMAX)
        for m in range(KMAX):
            if not a[m]:
                continue
            for n in range(KMAX):
                v = a[m] * b[n]
                if not v:
                    continue
                assert m + n < KMAX
                out[m + n] += v / 2
                out[abs(m - n)] += v / 2
        return out

    M = np.zeros((KMAX, KMAX))
    for k, op in SCHEME.items():
        if op[0] == "sin":
            M[k, 1] = 1.0
        elif op[0] == "square":
            M[k] = mul(M[op[1]] * SQRT2, M[op[1]] * SQRT2)
        elif op[0] == "mul":
            M[k] = mul(M[op[1]], M[op[2]])
        elif op[0] == "one":
            M[k, 0] = 1.0
    return M


def _build(reps: int = 1, loop: bool = False, cfg: dict | None = None):
    import concourse.tile as tile
    from concourse import bacc, mybir
    from concourse.bass import AP

    cfg = {**CFG, **(cfg or {})}
    f32 = mybir.dt.float32
    bf16 = mybir.dt.bfloat16
    AF = mybir.ActivationFunctionType
    OP = mybir.AluOpType

    nc = bacc.Bacc("TRN2", target_bir_lowering=False, debug=False)
    xx = nc.dram_tensor("xx", [128, 512], f32, kind="ExternalInput").ap()
    sout = nc.dram_tensor("sout", [80, 160], f32, kind="ExternalOutput").ap()

    unroll = cfg["unroll"] if loop else 1
    if loop:
        assert reps % unroll == 0, (reps, unroll)

    with tile.TileContext(nc) as tc:
        with tc.tile_pool(name="cpool", bufs=1) as cpool, \
             tc.tile_pool(name="pool", bufs=cfg["bufs"]) as pool, \
             tc.tile_pool(name="ppool", bufs=cfg["pbufs"],
                          space="PSUM") as ppool:
            scale_t = cpool.tile([128, 1], f32)     # -pi   (Sin input scale)
            bias_t = cpool.tile([128, 1], f32)      # pi/2  (Sin input bias)
            zero_t = cpool.tile([128, 1], f32)      # 0     (Square bias)
            nc.vector.memset(scale_t[:], -float(np.pi))
            nc.vector.memset(zero_t[:], 0.0)
            # warmup: loads the Sin table outside the loop; body activations
            # depend on its (zero) output via their bias tiles, so the
            # table load cannot sink into the loop.
            nc.scalar.activation(zero_t[:], zero_t[:], AF.Sin,
                                 bias=0.0, scale=1.0)
            nc.vector.tensor_scalar(bias_t[:], zero_t[:],
                                    0.5 * float(np.pi), None, OP.add)

            def body(_i=None):
                XX = pool.tile([128, 512], f32, tag="XX")
                C = pool.tile([128, 5120], bf16, tag="C")
                # layouts (q = dd*32 + c*2 + g indexes the 64 matmul blocks):
                #   XX col = q*8 + b'            (= dd*256 + c*16 + g*8 + b')
                #   C  col = q*80 + k*8 + b'     (k: mode plane)
                # every matmul operand C[:, q*80 : q*80+80] is contiguous.
                Xb = XX[:].rearrange("p (q b) -> p q b", q=64)
                Cb = C[:].rearrange("p (q k b) -> p q k b", q=64, k=KMAX)

                skip = cfg["skip"]
                nc.sync.dma_start(XX[:], xx)
                if "act" not in skip:
                    # p1 = sin(pi/2 - pi x) = cos(pi x); arg in (-pi/2, pi/2]
                    nc.scalar.activation(Cb[:, :, 1], Xb, AF.Sin,
                                         bias=bias_t[:], scale=scale_t[:])
                    # p2 = (sqrt2 c1)^2 = 1 + c2 ; p4 = (sqrt2 p2)^2
                    nc.scalar.activation(Cb[:, :, 2], Cb[:, :, 1], AF.Square,
                                         bias=zero_t[:], scale=SQRT2)
                    nc.scalar.activation(Cb[:, :, 4], Cb[:, :, 2], AF.Square,
                                         bias=zero_t[:], scale=SQRT2)
                if "tt" not in skip:
                    nc.vector.tensor_tensor(Cb[:, :, 6], Cb[:, :, 2],
                                            Cb[:, :, 4], OP.mult)
                    p8eng = nc.gpsimd if cfg["pool_p8"] else nc.vector
                    p8eng.tensor_tensor(Cb[:, :, 8], Cb[:, :, 4],
                                        Cb[:, :, 4], OP.mult)
                    if cfg["batched_tt"]:
                        # [p3,p5,p7,p9] = [p2,p4,p6,p8] * p1 (one DVE TT,
                        # stride-0 broadcast of the p1 sub-columns)
                        p1 = Cb[:, :, 1]
                        bc = AP(p1.tensor, p1.offset,
                                [p1.ap[0], p1.ap[1], [0, 4], p1.ap[2]])
                        nc.vector.tensor_tensor(Cb[:, :, 3:KMAX:2],
                                                Cb[:, :, 2:KMAX:2], bc,
                                                OP.mult)
                    else:
                        for k, i in ((3, 2), (5, 4), (7, 6), (9, 8)):
                            nc.vector.tensor_tensor(Cb[:, :, k], Cb[:, :, i],
                                                    Cb[:, :, 1], OP.mult)
                if "pool" not in skip:
                    nc.gpsimd.memset(Cb[:, :, 0], 1.0)

                ps = [ppool.tile([80, 80], f32, name=f"ps{g}", tag=f"ps{g}")
                      for g in range(2)]
                if "pe" not in skip:
                    for c in range(16):
                        for g in range(2):
                            q0 = c * 2 + g
                            nc.tensor.matmul(
                                ps[g][:],
                                C[:, q0 * 80:q0 * 80 + 80],
                                C[:, (32 + q0) * 80:(32 + q0) * 80 + 80],
                                start=(c == 0), stop=(c == 15))
                if "out" not in skip:
                    if cfg["psum_dma"]:
                        nc.sync.dma_start(sout[0:80, 0:80], ps[0][:])
                        nc.sync.dma_start(sout[0:80, 80:160], ps[1][:])
                    else:
                        SO = pool.tile([80, 160], f32, tag="SO")
                        # split PSUM evacuation across ACT and DVE
                        nc.scalar.copy(SO[0:80, 0:80], ps[0][:])
                        nc.vector.tensor_copy(SO[0:80, 80:160], ps[1][:])
                        nc.sync.dma_start(sout, SO[0:80, :])

            if loop:
                with tc.For_i(0, reps // unroll, 1) as i:
                    for _ in range(unroll):
                        body(i)
            else:
                for _ in range(reps):
                    body()

    nc.compile()
    return nc


def _get_state():
    if "nc" not in _STATE:
        _STATE["nc"] = _build()
    return _STATE["nc"]


def _shard_inputs(x: np.ndarray):
    """x [512, 16, 32, 2] -> per-core {xx [128, 512]}.

    xx free layout: dd*256 + c*16 + g*8 + b'  (q-blocks ordered (c, g);
    batch b = g*8 + b'). partition p = tp*32 + a for timestep-subgroup
    tp in 4, agent a in 32.
    """
    in_maps = []
    for core in range(NCORES):
        xc = x[core * TLOC:(core + 1) * TLOC]      # [64, 16, 32, 2]
        arr = xc.reshape(16, 4, 2, 8, 32, 2)       # (c, tp, g, b', a, d)
        arr = arr.transpose(5, 1, 4, 0, 2, 3)      # (d, tp, a, c, g, b')
        arr = arr.reshape(2, 128, 256)
        xxc = np.concatenate([arr[0], arr[1]], axis=1)
        in_maps.append({"xx": np.ascontiguousarray(xxc)})
    return in_maps


def _gather(souts):
    """souts: list of 8 [80, 160] partials -> scalar loss (float32).

    sout row = k0*8 + b', col (80*g + k1*8 + b'') for batch b = 8*g + b'.
    """
    total = np.zeros((80, 160), dtype=np.float64)
    for s in souts:
        total += s.astype(np.float64)
    St = np.empty((B, KMAX, KMAX), dtype=np.float64)
    for g in range(2):
        for bp in range(8):
            St[8 * g + bp] = total[bp::8, 80 * g + bp:80 * (g + 1):8]
    Minv = np.linalg.inv(_build_M())
    S = np.einsum("ki,bij,lj->bkl", Minv, St, Minv)
    nf, nw, cd = _np_constants()
    coeffs = S.reshape(B, KN) / (NA * T) / nf[None, :].astype(np.float64)
    d = nw[None, :].astype(np.float64) * (coeffs - cd[None, :].astype(np.float64))
    loss = np.mean(d * d)
    return np.float32(loss)


def kernel(x: np.ndarray) -> np.ndarray:
    from concourse.bass_utils import run_bass_kernel_spmd

    nc = _get_state()
    in_maps = _shard_inputs(np.asarray(x, dtype=np.float32))
    res = run_bass_kernel_spmd(nc, in_maps, list(range(NCORES)))
    souts = [r["sout"] for r in res.results]
    return _gather(souts)


# revision 5
# speedup vs baseline: 1.3251x; 1.0059x over previous
"""Trainium2 Bass kernel for the ergodicity loss (Chebyshev-cascade scheme).

Math: for x[T=512, B=16, N=32, d=2] in [0,1]^2 and modes (k0,k1) in {0..9}^2:
    basis = cos(pi*k0*x0) * cos(pi*k1*x1)                    (separable)
    coeffs[b, k0, k1] = sum_{t,n} basis / (T*N) / nf[k1]
    loss = mean((nw * (coeffs - cd))**2)

Device strategy (8 cores, data-parallel over T: 64 timesteps/core):
  Per core: 2048 points per batch as 128 partitions x 16 chunks, both
  coordinate dims side by side (xx[p, dd*256 + c*16 + b]).

  Mode columns (bf16) hold, per mode k, a KNOWN LINEAR COMBINATION of
  the true cosines c_j = cos(pi*j*x), built WITHOUT any range-reduction
  tricks via a Chebyshev product cascade seeded by one direct Sin:
    - p1 = Sin(-pi*x + pi/2) = cos(pi*x) = c1   (arg in (-pi/2, pi/2],
      inside the HW Sin table domain; ACT, reads xx f32 directly)
    - p2 = Square(sqrt2*p1) = 2*c1^2 = 1 + c2             (ACT)
    - p4 = Square(sqrt2*p2) = c4 + 4c2 + 3                (ACT)
    - p6 = p2*p4, p8 = p4*p4                              (DVE TT)
    - [p3,p5,p7,p9] = [p2,p4,p6,p8] * broadcast(p1): ONE batched DVE TT
      (stride-16 plane APs + stride-0 broadcast of p1)
    - p0 = 1                                              (Pool memset)
  The mode-k columns are thus M @ c for a constant triangular M
  (|Minv|max = 60; high-mode noise amplification is crushed by the
  k^-3 loss weights nw); the PSUM Gram matrix S~ = M S M^T is unmixed
  on the host with M^-1 (exact linear algebra, fp64).

  PE: per (chunk c in 16, batch-group g in 2): lhsT = dim-0 columns
  [k:10 x b:8] (contiguous 80-col block), rhs = dim-1 columns;
  accumulate into PSUM[80, 80] over the 16 chunks. Off-diagonal batch
  blocks unused.

  The activation-table load (1283 ns) is hoisted out of the bench loop
  by a pre-loop warmup Sin whose (zero) output feeds the bias tile
  every in-loop activation consumes.
Host: sum 8 per-core [80,160] PSUM dumps, extract diagonal batch
blocks, S = Minv S~ Minv^T, then the tiny [16,100] weighted MSE.
"""
import numpy as np

T, B, NA, D = 512, 16, 32, 2
KMAX = 10
NCORES = 8
TLOC = T // NCORES          # 64 timesteps per core
KN = KMAX * KMAX

SQRT2 = float(np.sqrt(2.0))

# plane construction DAG (dependency order):
#   ('sin',)            plane = cos(pi x)
#   ('square', src)     plane = (sqrt2 * p_src)^2
#   ('mul', i, j)       plane = p_i * p_j          (DVE TT)
#   ('one',)            plane = 1                  (Pool memset)
# [p3,p5,p7,p9] = [p2,p4,p6,p8]*p1 is emitted as ONE batched TT.
SCHEME = {
    1: ("sin",),
    2: ("square", 1),
    4: ("mul", 2, 2),       # Pool TT when cfg["p4_pool"], else ACT Square
    6: ("mul", 2, 4),
    8: ("mul", 4, 4),
    3: ("mul", 2, 1),
    5: ("mul", 4, 1),
    7: ("mul", 6, 1),
    9: ("mul", 8, 1),
    0: ("one",),
}

_STATE = {}

CFG = {"unroll": 32, "psum_dma": False, "bufs": 5, "pbufs": 4, "skip": (),
       "batched_tt": True, "pool_p8": False, "p4_pool": True, "so": "act"}


def _np_constants():
    """Replicates reference._constants() exactly in numpy (L = ones)."""
    L = np.ones(D, dtype=np.float32)
    grids = np.meshgrid(*[np.arange(KMAX) for _ in range(D)], indexing="ij")
    K = np.stack(grids, -1).reshape(-1, D).astype(np.float32)          # [100, 2]
    k_scaled = K * np.pi / L
    nf = np.where(K[:, -1] != 0, np.sqrt(L[-1] / 2.0), 1.0).astype(np.float32)
    nw = ((1.0 + (k_scaled ** 2).sum(-1)) ** (-(D + 1) / 2.0) * 100.0).astype(np.float32)
    safe_k = np.where(K != 0, k_scaled, 1.0)
    term = np.where(K != 0,
                    (np.exp(1j * k_scaled * L) - 1.0) / (1j * safe_k * L),
                    1.0 + 0j)
    cd = (term.prod(-1).real / nf).astype(np.float32)                  # [100]
    return nf, nw, cd


def _build_M():
    """M[k] = stored plane k as a linear combination of c_0..c_9.

    Symbolic expansion over the Chebyshev product rule:
    c_m * c_n = (c_{m+n} + c_{|m-n|}) / 2 (terms beyond c_9 are dropped:
    the construction never produces them for k<=9 products used here).
    """
    def mul(a, b):
        out = np.zeros(KMAX)
        for m in range(KMAX):
            if not a[m]:
                continue
            for n in range(KMAX):
                v = a[m] * b[n]
                if not v:
                    continue
                assert m + n < KMAX
                out[m + n] += v / 2
                out[abs(m - n)] += v / 2
        return out

    M = np.zeros((KMAX, KMAX))
    for k, op in SCHEME.items():
        if op[0] == "sin":
            M[k, 1] = 1.0
        elif op[0] == "square":
            M[k] = mul(M[op[1]] * SQRT2, M[op[1]] * SQRT2)
        elif op[0] == "mul":
            M[k] = mul(M[op[1]], M[op[2]])
        elif op[0] == "one":
            M[k, 0] = 1.0
    return M


def _build(reps: int = 1, loop: bool = False, cfg: dict | None = None):
    import concourse.tile as tile
    from concourse import bacc, mybir
    from concourse.bass import AP

    cfg = {**CFG, **(cfg or {})}
    f32 = mybir.dt.float32
    bf16 = mybir.dt.bfloat16
    AF = mybir.ActivationFunctionType
    OP = mybir.AluOpType

    nc = bacc.Bacc("TRN2", target_bir_lowering=False, debug=False)
    xx = nc.dram_tensor("xx", [128, 512], f32, kind="ExternalInput").ap()
    sout = nc.dram_tensor("sout", [80, 160], f32, kind="ExternalOutput").ap()

    unroll = cfg["unroll"] if loop else 1
    if loop:
        assert reps % unroll == 0, (reps, unroll)

    with tile.TileContext(nc) as tc:
        with tc.tile_pool(name="cpool", bufs=1) as cpool, \
             tc.tile_pool(name="pool", bufs=cfg["bufs"]) as pool, \
             tc.tile_pool(name="ppool", bufs=cfg["pbufs"],
                          space="PSUM") as ppool:
            scale_t = cpool.tile([128, 1], f32)     # -pi   (Sin input scale)
            bias_t = cpool.tile([128, 1], f32)      # pi/2  (Sin input bias)
            zero_t = cpool.tile([128, 1], f32)      # 0     (Square bias)
            nc.vector.memset(scale_t[:], -float(np.pi))
            nc.vector.memset(zero_t[:], 0.0)
            # warmup: loads the Sin table outside the loop; body activations
            # depend on its (zero) output via their bias tiles, so the
            # table load cannot sink into the loop.
            nc.scalar.activation(zero_t[:], zero_t[:], AF.Sin,
                                 bias=0.0, scale=1.0)
            nc.vector.tensor_scalar(bias_t[:], zero_t[:],
                                    0.5 * float(np.pi), None, OP.add)

            def body(_i=None):
                XX = pool.tile([128, 512], f32, tag="XX")
                C = pool.tile([128, 5120], bf16, tag="C")
                # layouts (q = dd*32 + c*2 + g indexes the 64 matmul blocks):
                #   XX col = q*8 + b'            (= dd*256 + c*16 + g*8 + b')
                #   C  col = q*80 + k*8 + b'     (k: mode plane)
                # every matmul operand C[:, q*80 : q*80+80] is contiguous.
                Xb = XX[:].rearrange("p (q b) -> p q b", q=64)
                Cb = C[:].rearrange("p (q k b) -> p q k b", q=64, k=KMAX)

                skip = cfg["skip"]
                nc.sync.dma_start(XX[:], xx)
                if "act" not in skip:
                    # p1 = sin(pi/2 - pi x) = cos(pi x); arg in (-pi/2, pi/2]
                    nc.scalar.activation(Cb[:, :, 1], Xb, AF.Sin,
                                         bias=bias_t[:], scale=scale_t[:])
                    # p2 = (sqrt2 c1)^2 = 1 + c2
                    nc.scalar.activation(Cb[:, :, 2], Cb[:, :, 1], AF.Square,
                                         bias=zero_t[:], scale=SQRT2)
                    if cfg["p4_pool"]:
                        # p4 = p2^2 on the otherwise-idle Pool engine
                        nc.gpsimd.tensor_tensor(Cb[:, :, 4], Cb[:, :, 2],
                                                Cb[:, :, 2], OP.mult)
                    else:
                        nc.scalar.activation(Cb[:, :, 4], Cb[:, :, 2],
                                             AF.Square,
                                             bias=zero_t[:], scale=SQRT2)
                if "tt" not in skip:
                    nc.vector.tensor_tensor(Cb[:, :, 6], Cb[:, :, 2],
                                            Cb[:, :, 4], OP.mult)
                    p8eng = nc.gpsimd if cfg["pool_p8"] else nc.vector
                    p8eng.tensor_tensor(Cb[:, :, 8], Cb[:, :, 4],
                                        Cb[:, :, 4], OP.mult)
                    if cfg["batched_tt"]:
                        # [p3,p5,p7,p9] = [p2,p4,p6,p8] * p1 (one DVE TT,
                        # stride-0 broadcast of the p1 sub-columns)
                        p1 = Cb[:, :, 1]
                        bc = AP(p1.tensor, p1.offset,
                                [p1.ap[0], p1.ap[1], [0, 4], p1.ap[2]])
                        nc.vector.tensor_tensor(Cb[:, :, 3:KMAX:2],
                                                Cb[:, :, 2:KMAX:2], bc,
                                                OP.mult)
                    else:
                        for k, i in ((3, 2), (5, 4), (7, 6), (9, 8)):
                            nc.vector.tensor_tensor(Cb[:, :, k], Cb[:, :, i],
                                                    Cb[:, :, 1], OP.mult)
                if "pool" not in skip:
                    nc.gpsimd.memset(Cb[:, :, 0], 1.0)

                ps = [ppool.tile([80, 80], f32, name=f"ps{g}", tag=f"ps{g}")
                      for g in range(2)]
                if "pe" not in skip:
                    for c in range(16):
                        for g in range(2):
                            q0 = c * 2 + g
                            nc.tensor.matmul(
                                ps[g][:],
                                C[:, q0 * 80:q0 * 80 + 80],
                                C[:, (32 + q0) * 80:(32 + q0) * 80 + 80],
                                start=(c == 0), stop=(c == 15))
                if "out" not in skip:
                    SO = pool.tile([80, 160], f32, tag="SO")
                    if cfg["so"] == "act":
                        nc.scalar.copy(SO[0:80, 0:80], ps[0][:])
                        nc.scalar.copy(SO[0:80, 80:160], ps[1][:])
                    elif cfg["so"] == "dve":
                        nc.vector.tensor_copy(SO[0:80, 0:80], ps[0][:])
                        nc.vector.tensor_copy(SO[0:80, 80:160], ps[1][:])
                    else:  # split PSUM evacuation across ACT and DVE
                        nc.scalar.copy(SO[0:80, 0:80], ps[0][:])
                        nc.vector.tensor_copy(SO[0:80, 80:160], ps[1][:])
                    nc.sync.dma_start(sout, SO[0:80, :])

            if loop:
                with tc.For_i(0, reps // unroll, 1) as i:
                    for _ in range(unroll):
                        body(i)
            else:
                for _ in range(reps):
                    body()

    nc.compile()
    return nc


def _get_state():
    if "nc" not in _STATE:
        _STATE["nc"] = _build()
    return _STATE["nc"]


def _shard_inputs(x: np.ndarray):
    """x [512, 16, 32, 2] -> per-core {xx [128, 512]}.

    xx free layout: dd*256 + c*16 + g*8 + b'  (q-blocks ordered (c, g);
    batch b = g*8 + b'). partition p = tp*32 + a for timestep-subgroup
    tp in 4, agent a in 32.
    """
    in_maps = []
    for core in range(NCORES):
        xc = x[core * TLOC:(core + 1) * TLOC]      # [64, 16, 32, 2]
        arr = xc.reshape(16, 4, 2, 8, 32, 2)       # (c, tp, g, b', a, d)
        arr = arr.transpose(5, 1, 4, 0, 2, 3)      # (d, tp, a, c, g, b')
        arr = arr.reshape(2, 128, 256)
        xxc = np.concatenate([arr[0], arr[1]], axis=1)
        in_maps.append({"xx": np.ascontiguousarray(xxc)})
    return in_maps


def _gather(souts):
    """souts: list of 8 [80, 160] partials -> scalar loss (float32).

    sout row = k0*8 + b', col (80*g + k1*8 + b'') for batch b = 8*g + b'.
    """
    total = np.zeros((80, 160), dtype=np.float64)
    for s in souts:
        total += s.astype(np.float64)
    St = np.empty((B, KMAX, KMAX), dtype=np.float64)
    for g in range(2):
        for bp in range(8):
            St[8 * g + bp] = total[bp::8, 80 * g + bp:80 * (g + 1):8]
    Minv = np.linalg.inv(_build_M())
    S = np.einsum("ki,bij,lj->bkl", Minv, St, Minv)
    nf, nw, cd = _np_constants()
    coeffs = S.reshape(B, KN) / (NA * T) / nf[None, :].astype(np.float64)
    d = nw[None, :].astype(np.float64) * (coeffs - cd[None, :].astype(np.float64))
    loss = np.mean(d * d)
    return np.float32(loss)


def kernel(x: np.ndarray) -> np.ndarray:
    from concourse.bass_utils import run_bass_kernel_spmd

    nc = _get_state()
    in_maps = _shard_inputs(np.asarray(x, dtype=np.float32))
    res = run_bass_kernel_spmd(nc, in_maps, list(range(NCORES)))
    souts = [r["sout"] for r in res.results]
    return _gather(souts)
